# revision 28
# baseline (speedup 1.0000x reference)
"""Trainium2 Bass kernel for nn_CGT_21354577396059 (GPS-style GNN, 3 streams x 3 layers).

Strategy (8 NeuronCores, SPMD):
- Node-shard: core c owns nodes [2048c, 2048c+2048) = 8 graphs of 256 nodes.
- Activations feature-major in SBUF: hT [C=128 partitions, 2048 nodes] fp32,
  bf16 copies as matmul inputs (fp32 matmul is 4x slower on the PE).
- GIN segment_sum: edges dst-sorted per core, padded per 128-node window to a
  uniform chunk capacity; src rows gathered from a bf16 node-major DRAM table
  (gpsimd dma_gather, 256B rows); scatter via one-hot matmuls
  aggT += gathered_chunk.T @ onehot_chunk (PE, fp32 PSUM accumulate).
- The bf16 node table is rebuilt each layer via PE transpose + 8-core AllGather.
- Attention is graph-local: scoresT = kT.T @ qT per (graph, head, key-chunk)
  with row-packed tile_position; exp on ACT (no max subtraction; |scores|<~50);
  softmax sums via col-tiled ones-matmuls; o via col-tiled matmuls contracted
  over keys; normalization with a DVE reciprocal + elementwise mul.

kernel(**inputs) takes the FULL unsharded inputs and returns
(pool(h0), pool(ha), pool(hb)) — tuple of [64, 128] float32 — like the reference.
"""
import sys
import numpy as np
import ml_dtypes

if "/opt/trn_rl_repo" not in sys.path:
    sys.path.insert(0, "/opt/trn_rl_repo")

import os
import concourse.bass as bass  # noqa: F401
import concourse.tile as tile
from concourse import bacc, mybir, library_config
from concourse.bass_utils import run_bass_kernel_spmd

BF = ml_dtypes.bfloat16

# Problem constants (self-contained; no reads of /root/problem/*)
N_NODES = 16384
N_GRAPHS = 64
NPG = 256
FEA_DIM = 32
PE_DIM = 20
C = 128
HEADS = 4
HD = C // HEADS
L = 3
BN_EPS = 1e-5
S_BN = float(1.0 / np.sqrt(1.0 + BN_EPS))

N_CORES = 8
NPC = N_NODES // N_CORES   # 2048
NG_C = NPC // NPG          # 8 graphs per core
WIN = 128
NWIN = NPC // WIN          # 16
NCHUNK = NPC // 128        # 16
WGRP = 2                   # windows per dma_gather call

fdt = mybir.dt.float32
bdt = mybir.dt.bfloat16
i16 = mybir.dt.int16
AF = mybir.ActivationFunctionType
AX = mybir.AxisListType
ALU = mybir.AluOpType
AG_GROUPS = [list(range(N_CORES))]


# ---------------------------------------------------------------------------
# Host-side data prep
# ---------------------------------------------------------------------------

def _wrap_idxs(idx):
    """dma_gather idx layout [128, n/16] int16: idx i at (i%16, i//16),
    replicated across the 8 16-partition blocks."""
    n = len(idx)
    a = np.asarray(idx, np.int16).reshape(n // 16, 16).T
    return np.ascontiguousarray(np.tile(a, (8, 1)))


def _prep_edges_stream(edge_index):
    """Returns (cap_chunks, [(gidx_wrapped, dstv_bf16)] per core).

    Edges within each 128-dst window are sorted by src for HBM row-buffer
    locality during the gather. dstv[p, w*cap+t] is the window-local dst of
    edge slot t*128+p in window w (-1 for padding); the one-hot scatter
    matrices are generated on-chip from it."""
    src = np.asarray(edge_index[0]).astype(np.int64)
    dst = np.asarray(edge_index[1]).astype(np.int64)
    per_core_wins = []
    max_w = 0
    for c in range(N_CORES):
        m = (dst >= c * NPC) & (dst < (c + 1) * NPC)
        s, d = src[m], dst[m] - c * NPC
        order = np.argsort(d, kind="stable")
        s, d = s[order], d[order]
        wins = []
        for w in range(NWIN):
            mw = (d >= w * WIN) & (d < (w + 1) * WIN)
            sw, dw = s[mw], d[mw] - w * WIN
            so = np.argsort(sw, kind="stable")
            wins.append((sw[so], dw[so]))
            max_w = max(max_w, int(mw.sum()))
        per_core_wins.append(wins)
    cap_e = ((max_w + 127) // 128) * 128     # edges per window, padded
    cap = cap_e // 128
    out = []
    for c in range(N_CORES):
        srcs = np.zeros(NWIN * cap_e, np.int64)
        dstv = np.full((NWIN * cap_e,), -1.0, np.float32)
        for w in range(NWIN):
            s, dloc = per_core_wins[c][w]
            n = len(s)
            srcs[w * cap_e:w * cap_e + n] = s
            dstv[w * cap_e + np.arange(n)] = dloc
        dv = dstv.reshape(NWIN, cap, 128).transpose(2, 0, 1).reshape(128, NWIN * cap)
        out.append((_wrap_idxs(srcs), np.ascontiguousarray(dv.astype(BF))))
    return cap, out


def _pack_host(inputs):
    inp = {k: np.asarray(v) for k, v in inputs.items()}
    rt2 = 1.0 / np.sqrt(HD)

    blocks, offs = [], {}

    def add(name, arr):
        arr = np.asarray(arr, np.float32)
        k, m = arr.shape
        buf = np.zeros((128, m), BF)
        buf[:k] = arr.astype(BF)
        offs[name] = sum(b.shape[1] for b in blocks)
        blocks.append(buf)

    add("emb", inp["node_emb_w"])
    add("pe", inp["pe_lin_w"])
    add("I2", 2.0 * np.eye(C))       # h2 fold: ACT scale s gives 2s*h
    add("I1", np.eye(C))             # transpose identity + acc fold
    add("ones32", np.ones((C, HD)))
    add("iota", np.tile(np.arange(C, dtype=np.float32)[None, :], (C, 1)))
    for l in range(L):
        aw = inp["attn_in_w"][l]
        add(f"gw1_{l}", inp["gin_w1"][l])
        add(f"gw2_{l}", inp["gin_w2"][l])
        add(f"qT_{l}", (aw[0:C] * rt2).T)
        for h in range(HEADS):
            km = aw[C:2 * C].T.copy()
            mask = np.zeros(C); mask[32 * h:32 * h + 32] = 1.0
            add(f"kTm{h}_{l}", km * mask[None, :])
        add(f"vT_{l}", aw[2 * C:3 * C].T)
        add(f"ow_{l}", inp["attn_out_w"][l].T)
        add(f"m1_{l}", inp["mlp_w1"][l])
        add(f"m2a_{l}", inp["mlp_w2"][l][0:C])
        add(f"m2b_{l}", inp["mlp_w2"][l][C:2 * C])
    wts = np.ascontiguousarray(np.concatenate(blocks, axis=1))

    bvecs, boffs = [], {}

    def addb(name, vec):
        vec = np.asarray(vec, np.float32).reshape(-1)
        assert vec.shape == (C,)
        boffs[name] = len(bvecs)
        bvecs.append(vec)

    addb("eb", inp["node_emb_b"] + inp["pe_lin_b"])
    for l in range(L):
        ab = inp["attn_in_b"][l]
        addb(f"gb1_{l}", inp["gin_b1"][l])
        addb(f"sgb2_{l}", S_BN * inp["gin_b2"][l])
        addb(f"qb_{l}", ab[0:C] * rt2)
        for h in range(HEADS):
            mask = np.zeros(C); mask[32 * h:32 * h + 32] = 1.0
            addb(f"kbm{h}_{l}", ab[C:2 * C] * mask)
        addb(f"sob_{l}", S_BN * inp["attn_out_b"][l])
        addb(f"m1ba_{l}", inp["mlp_b1"][l][0:C])
        addb(f"m1bb_{l}", inp["mlp_b1"][l][C:2 * C])
        addb(f"smb2_{l}", S_BN * inp["mlp_b2"][l])
    biases = np.ascontiguousarray(np.stack(bvecs, axis=1).astype(np.float32))

    vbias = np.ascontiguousarray(np.stack(
        [np.tile(inp["attn_in_b"][l][2 * C:3 * C], (128, 1)) for l in range(L)]
    ).astype(np.float32))

    caps, edges = [], []
    for key in ("edge_index", "edge_index1", "edge_index2"):
        cap_chunks, per_core = _prep_edges_stream(inp[key])
        caps.append(cap_chunks)
        edges.append(per_core)

    xs = [inp["x"], inp["x1"], inp["x2"]]
    in_maps = []
    for c in range(N_CORES):
        m = {"wts": wts, "biases": biases, "vbias": vbias}
        sl = slice(c * NPC, (c + 1) * NPC)
        for s in range(3):
            m[f"xT{s}"] = np.ascontiguousarray(xs[s][sl].T.astype(BF))
            m[f"gidx{s}"] = edges[s][c][0]
            m[f"dstv{s}"] = edges[s][c][1]
        m["peT"] = np.ascontiguousarray(inp["pe"][sl].T.astype(BF))
        in_maps.append(m)

    return in_maps, caps, offs, boffs, wts.shape[1], biases.shape[1]


# ---------------------------------------------------------------------------
# Kernel builder
# ---------------------------------------------------------------------------


def _mm(nc, out, lhsT, rhs, start, stop, nmax=512):
    """matmul with moving free dim split to <=512 (ISA limit)."""
    n = rhs.shape[-1]
    assert out.shape[-1] == n
    for i in range(0, n, nmax):
        j = min(i + nmax, n)
        nc.tensor.matmul(out[:, i:j], lhsT, rhs[:, i:j], start=start, stop=stop)


def build_kernel(caps, offs, boffs, wcols, bcols, n_layers=L, n_streams=3,
                 dbg_stream=None):
    nc = bacc.Bacc("TRN2", target_bir_lowering=False, num_devices=N_CORES,
                   num_swdge_queues=4)

    wts_d = nc.dram_tensor("wts", [128, wcols], bdt, kind="ExternalInput")
    bias_d = nc.dram_tensor("biases", [128, bcols], fdt, kind="ExternalInput")
    vbias_d = nc.dram_tensor("vbias", [L, 128, 128], fdt, kind="ExternalInput")
    peT_d = nc.dram_tensor("peT", [PE_DIM, NPC], bdt, kind="ExternalInput")
    xT_d, gidx_d, dstv_d = [], [], []
    for s in range(3):
        cap = caps[s]
        xT_d.append(nc.dram_tensor(f"xT{s}", [FEA_DIM, NPC], bdt,
                                   kind="ExternalInput"))
        gidx_d.append(nc.dram_tensor(f"gidx{s}", [128, NWIN * cap * 8], i16,
                                     kind="ExternalInput"))
        dstv_d.append(nc.dram_tensor(f"dstv{s}", [128, NWIN * cap], bdt,
                                     kind="ExternalInput"))
    pool_out = nc.dram_tensor("pool_out", [3, C, NG_C], fdt,
                              kind="ExternalOutput")
    dbg_out = None
    if dbg_stream is not None:
        dbg_out = nc.dram_tensor("dbg_out", [C, NPC], fdt, kind="ExternalOutput")

    with tile.TileContext(nc) as tc:
        with (
            tc.tile_pool(name="const", bufs=1) as const_p,
            tc.tile_pool(name="hstate", bufs=1) as hstate_p,
            tc.tile_pool(name="big", bufs=1) as big_p,       # full-width tiles
            tc.tile_pool(name="chunk", bufs=2) as chunk_p,   # [C,1024]-ish tiles
            tc.tile_pool(name="gath", bufs=2) as gath_p,
            tc.tile_pool(name="ohp", bufs=2) as oh_p,
            tc.tile_pool(name="ps2", bufs=3, space="PSUM") as ps2,   # 2-bank
            tc.tile_pool(name="ps1", bufs=2, space="PSUM") as ps1,   # 1-bank
            tc.tile_pool(name="dram", bufs=4, space="DRAM") as dram_p,
        ):
            wts = const_p.tile([128, wcols], bdt)
            nc.sync.dma_start(wts[:], wts_d[:])
            bia = const_p.tile([128, bcols], fdt)
            nc.sync.dma_start(bia[:], bias_d[:])
            vbia = const_p.tile([128, L, 128], fdt)
            nc.sync.dma_start(vbia[:], vbias_d[:].rearrange("l p c -> p l c"))

            def W(name, width=128, rows=128):
                return wts[0:rows, offs[name]:offs[name] + width]

            def B(name):
                return bia[:, boffs[name]:boffs[name] + 1]

            hT = [hstate_p.tile([C, NPC], fdt, tag=f"hT{s}", name=f"hT{s}")
                  for s in range(3)]
            hB = [hstate_p.tile([C, NPC], bdt, tag=f"hB{s}", name=f"hB{s}")
                  for s in range(3)]
            gidx = []
            dstv = []
            for s in range(n_streams):
                t = hstate_p.tile([128, NWIN * caps[s] * 8], i16,
                                  tag=f"gidx{s}", name=f"gidx{s}")
                nc.sync.dma_start(t[:], gidx_d[s][:])
                gidx.append(t)
                dv = hstate_p.tile([128, NWIN * caps[s]], bdt,
                                   tag=f"dstv{s}", name=f"dstv{s}")
                nc.sync.dma_start(dv[:], dstv_d[s][:])
                dstv.append(dv)

            def emit_table(src_bf, s):
                """feature-major bf16 [C, NPC] -> node-major -> AllGather table."""
                nm = big_p.tile([128, NCHUNK, C], bdt, tag="nm")
                for k in range(NCHUNK):
                    pt = ps1.tile([128, 128], bdt, tag="ps1", name="pt")
                    nc.tensor.transpose(pt[:], src_bf[:, k * 128:(k + 1) * 128],
                                        W("I1"))
                    nc.vector.tensor_copy(nm[:, k, :], pt[:])
                agi = dram_p.tile([NPC, C], bdt, tag="agi")
                nc.sync.dma_start(agi[:].rearrange("(k p) c -> p k c", p=128),
                                  nm[:])
                tab = dram_p.tile([N_NODES, C], bdt, tag=f"tab{s}", name="tab",
                                  bufs=1, addr_space="Shared")
                nc.gpsimd.collective_compute(
                    "AllGather", ALU.bypass, replica_groups=AG_GROUPS,
                    ins=[agi.opt()], outs=[tab.opt()])
                return tab

            # ---------------- embedding ----------------
            tabs = [None] * 3
            for s in range(n_streams):
                xT = chunk_p.tile([FEA_DIM, NPC], bdt, tag="xT", bufs=1)
                nc.sync.dma_start(xT[:], xT_d[s][:])
                peT = chunk_p.tile([PE_DIM, NPC], bdt, tag="peT", bufs=1)
                nc.sync.dma_start(peT[:], peT_d[:])
                for ch in range(2):
                    sl = slice(ch * 1024, (ch + 1) * 1024)
                    ep = ps2.tile([C, 1024], fdt, tag="ps2")
                    _mm(nc, ep, W("emb", rows=FEA_DIM), xT[:, sl],
                        start=True, stop=False)
                    _mm(nc, ep, W("pe", rows=PE_DIM), peT[:, sl],
                        start=False, stop=True)
                    nc.scalar.activation(hT[s][:, sl], ep[:], AF.Identity,
                                         bias=B("eb"))
                    nc.vector.tensor_copy(hB[s][:, sl], hT[s][:, sl])
                tabs[s] = emit_table(hB[s], s)

            # ---------------- layers ----------------
            for l in range(n_layers):
                for s in range(n_streams):
                    _gps_layer(nc, l, caps[s], hT[s], hB[s], tabs, s,
                               gidx[s], dstv[s], W, B, vbia,
                               big_p, chunk_p, gath_p, oh_p, ps2, ps1,
                               emit_table, last=(l == n_layers - 1))

            # ---------------- pooling ----------------
            for s in range(n_streams):
                po = chunk_p.tile([C, NG_C], fdt, tag="pool")
                nc.vector.reduce_sum(
                    out=po[:],
                    in_=hT[s][:].rearrange("c (g n) -> c g n", g=NG_C),
                    axis=AX.X)
                nc.sync.dma_start(pool_out[s], po[:])

            if dbg_stream is not None:
                nc.sync.dma_start(dbg_out[:], hT[dbg_stream][:])

    nc.compile()
    return nc


def _gps_layer(nc, l, cap, hT, hB, tabs, s, gidx, dstv, W, B, vbia,
               big_p, chunk_p, gath_p, oh_p, ps2, ps1, emit_table, last):
    AFI = AF.Identity
    K_ATTN = True
    K_GIN = True
    K_GATHER = True
    K_SCORES = True
    K_SUMS = True
    K_OV = True

    # ---------------- attention (graph-local) ----------------
    if not K_ATTN:
        o_bf = big_p.tile([C, NPC], bdt, tag="o_bf")
        nc.vector.memset(o_bf[:], 0.0)
    qT = big_p.tile([C, NPC], bdt, tag="qT", name="qT") if K_ATTN else None
    for ch in range(2 if K_ATTN else 0):
        sl = slice(ch * 1024, (ch + 1) * 1024)
        qp = ps2.tile([C, 1024], fdt, tag="ps2")
        _mm(nc, qp, W(f"qT_{l}"), hB[:, sl], start=True, stop=True)
        nc.scalar.activation(qT[:, sl], qp[:], AFI, bias=B(f"qb_{l}"))
    v_sb = big_p.tile([128, NCHUNK, C], bdt, tag="v_sb", name="v_sb") if K_ATTN else None
    for k in range(NCHUNK if K_ATTN else 0):
        vp = ps1.tile([128, C], fdt, tag="ps1")
        nc.tensor.matmul(vp[:], hB[:, k * 128:(k + 1) * 128], W(f"vT_{l}"),
                         start=True, stop=True)
        nc.vector.tensor_add(v_sb[:, k, :], vp[:], vbia[:, l, :])

    expT = big_p.tile([128, 2, HEADS, NG_C, NPG], bdt, tag="expT", name="expT") if K_ATTN else None
    if K_ATTN and not K_SCORES:
        nc.vector.memset(expT[:], 1.0)
    for h in range(HEADS if (K_ATTN and K_SCORES) else 0):
        kTm = big_p.tile([C, NPC], bdt, tag="kTm", name="kTm")
        for ch in range(2):
            sl = slice(ch * 1024, (ch + 1) * 1024)
            kp = ps2.tile([C, 1024], fdt, tag="ps2")
            _mm(nc, kp, W(f"kTm{h}_{l}"), hB[:, sl], start=True, stop=True)
            nc.scalar.activation(kTm[:, sl], kp[:], AFI, bias=B(f"kbm{h}_{l}"))
        for kc in range(2):
            for gh in range(2):
                sp = ps2.tile([128, 4, NPG], fdt, tag="ps2")
                for gi in range(4):
                    g = gh * 4 + gi
                    ksl = kTm[:, g * NPG + kc * 128:g * NPG + kc * 128 + 128]
                    qsl = qT[:, g * NPG:(g + 1) * NPG]
                    nc.tensor.matmul(sp[:, gi, :], ksl, qsl,
                                     start=True, stop=True)
                nc.scalar.activation(expT[:, kc, h, gh * 4:gh * 4 + 4, :],
                                     sp[:], AF.Exp)

    recip = big_p.tile([128, NPC], bdt, tag="recip", name="recip") if K_ATTN else None
    if K_ATTN and not K_SUMS:
        nc.vector.memset(recip[:], 1.0 / 256.0)
    for half in range(2 if (K_ATTN and K_SUMS) else 0):
        sm = ps2.tile([128, 1024], fdt, tag="ps2")
        for h in range(HEADS):
            for qc in range(2):
                osl = sm[32 * h:32 * h + 32, qc * 512:(qc + 1) * 512]
                for kc in range(2):
                    rhs = expT[:, kc, h, :, :].rearrange("p g q -> p (g q)")
                    nc.tensor.matmul(
                        osl, W("ones32", width=HD),
                        rhs[:, half * 1024 + qc * 512:half * 1024 + (qc + 1) * 512],
                        start=(kc == 0), stop=(kc == 1),
                        tile_position=(0, 32 * h))
        with nc.allow_low_precision(reason="softmax recip bf16 ok at 2e-2 gate"):
            nc.vector.reciprocal(recip[:, half * 1024:(half + 1) * 1024], sm[:])

    if K_ATTN:
        o_bf = big_p.tile([C, NPC], bdt, tag="o_bf", name="o_bf")
        if not K_OV:
            nc.vector.memset(o_bf[:], 0.0)
    for g in range(NG_C if (K_ATTN and K_OV) else 0):
        op = ps1.tile([128, NPG], fdt, tag="ps1")
        for h in range(HEADS):
            for kc in range(2):
                lhs = v_sb[:, g * 2 + kc, 32 * h:32 * h + 32]
                rhs = expT[:, kc, h, g, :]
                nc.tensor.matmul(op[32 * h:32 * h + 32, :], lhs, rhs,
                                 start=(kc == 0), stop=(kc == 1),
                                 tile_position=(0, 32 * h))
        nc.vector.tensor_mul(o_bf[:, g * NPG:(g + 1) * NPG], op[:],
                             recip[:, g * NPG:(g + 1) * NPG])

    # ---------------- GIN ----------------
    g_bf = big_p.tile([C, NPC], bdt, tag="g_bf")
    if not K_GIN:
        nc.vector.tensor_copy(g_bf[:], hT[:])
    tab = tabs[s]
    cap_e = cap * 128                       # edges per window
    GMAX = 1024   # HW limit per dma_gather call
    qn = (l * 3 + s) % 4
    for wg in range(NWIN // WGRP if K_GIN else 0):
        nidx = WGRP * cap_e
        gath = gath_p.tile([128, WGRP * cap, C], bdt, tag="gath")
        base = wg * nidx
        if K_GATHER:
            off = 0
            while off < nidx:
                n = min(GMAX, nidx - off)
                isl = gidx[:, (base + off) // 16:(base + off + n) // 16]
                nc.gpsimd.dma_gather(
                    gath[:, off // 128:(off + n) // 128, :], tab[:], isl,
                    n, n, C, queue_num=qn)
                qn = (qn + 1) % 4
                off += n
        else:
            nc.vector.memset(gath[:], 0.0)
        for wi in range(WGRP):
            w = wg * WGRP + wi
            ohs = oh_p.tile([128, cap, WIN], bdt, tag="ohs")
            io = W("iota")
            b0 = bass.AP(io.tensor, io.offset,
                         [list(io.ap[0]), [0, cap], list(io.ap[1])])
            dv = dstv[:, w * cap:(w + 1) * cap]
            b1 = bass.AP(dv.tensor, dv.offset,
                         [list(dv.ap[0]), list(dv.ap[1]), [0, WIN]])
            nc.vector.tensor_tensor(ohs[:], b0, b1, ALU.is_equal)
            ap = ps1.tile([C, WIN], fdt, tag="ps1")
            for t in range(cap):
                nc.tensor.matmul(ap[:], gath[:, wi * cap + t, :], ohs[:, t, :],
                                 start=(t == 0), stop=(t == cap - 1))
            nc.vector.tensor_add(g_bf[:, w * WIN:(w + 1) * WIN], ap[:],
                                 hT[:, w * WIN:(w + 1) * WIN])

    # GIN MLP + combine with attention: acc = h1 + h2
    acc_bf = big_p.tile([C, NPC], bdt, tag="acc_bf")
    r_bf = big_p.tile([C, NPC], bdt, tag="r_bf")
    for ch in range(2):
        sl = slice(ch * 1024, (ch + 1) * 1024)
        tp = ps2.tile([C, 1024], fdt, tag="ps2")
        _mm(nc, tp, W(f"gw1_{l}"), g_bf[:, sl], start=True, stop=True)
        nc.scalar.activation(r_bf[:, sl], tp[:], AF.Relu, bias=B(f"gb1_{l}"))
        up = ps2.tile([C, 1024], fdt, tag="ps2")
        _mm(nc, up, W(f"gw2_{l}"), r_bf[:, sl], start=True, stop=True)
        h1 = chunk_p.tile([C, 1024], bdt, tag="h1")
        nc.scalar.activation(h1[:], up[:], AFI, bias=B(f"sgb2_{l}"), scale=S_BN)
        ap2 = ps2.tile([C, 1024], fdt, tag="ps2")
        _mm(nc, ap2, W(f"ow_{l}"), o_bf[:, sl], start=True, stop=False)
        _mm(nc, ap2, W("I2"), hB[:, sl], start=False, stop=True)
        h2 = chunk_p.tile([C, 1024], bdt, tag="h2")
        nc.scalar.activation(h2[:], ap2[:], AFI, bias=B(f"sob_{l}"), scale=S_BN)
        nc.vector.tensor_add(acc_bf[:, sl], h1[:], h2[:])

    # MLP (chunked: r2 hidden kept per-1024-chunk to save SBUF)
    for ch in range(2):
        sl = slice(ch * 1024, (ch + 1) * 1024)
        r2c = chunk_p.tile([C, 2, 1024], bdt, tag="r2c", bufs=1)
        for mh in range(2):
            mp = ps2.tile([C, 1024], fdt, tag="ps2")
            _mm(nc, mp, W(f"m1_{l}", width=256)[:, mh * 128:(mh + 1) * 128],
                acc_bf[:, sl], start=True, stop=True)
            bname = f"m1ba_{l}" if mh == 0 else f"m1bb_{l}"
            nc.scalar.activation(r2c[:, mh, :], mp[:], AF.Relu, bias=B(bname))
        m2p = ps2.tile([C, 1024], fdt, tag="ps2")
        _mm(nc, m2p, W(f"m2a_{l}"), r2c[:, 0, :], start=True, stop=False)
        _mm(nc, m2p, W(f"m2b_{l}"), r2c[:, 1, :], start=False, stop=False)
        _mm(nc, m2p, W("I1"), acc_bf[:, sl], start=False, stop=True)
        dh = chunk_p.tile([C, 1024], bdt, tag="dh")
        nc.scalar.activation(dh[:], m2p[:], AFI, bias=B(f"smb2_{l}"), scale=S_BN)
        nc.vector.tensor_add(hT[:, sl], hT[:, sl], dh[:])
        nc.vector.tensor_copy(hB[:, sl], hT[:, sl])

    if not last:
        tabs[s] = emit_table(hB, s)


# ---------------------------------------------------------------------------
# Entry point
# ---------------------------------------------------------------------------

_CACHE = {}


def _get_kernel(caps, offs, boffs, wcols, bcols, **kw):
    key = (tuple(caps), wcols, bcols, tuple(sorted(kw.items())))
    if key not in _CACHE:
        _CACHE[key] = build_kernel(caps, offs, boffs, wcols, bcols, **kw)
    return _CACHE[key]


def kernel(**inputs):
    in_maps, caps, offs, boffs, wcols, bcols = _pack_host(inputs)
    nc = _get_kernel(caps, offs, boffs, wcols, bcols)
    res = run_bass_kernel_spmd(nc, in_maps, core_ids=list(range(N_CORES)))
    pools = []
    for si in range(3):
        parts = [np.asarray(res.results[c]["pool_out"][si])
                 for c in range(N_CORES)]
        full = np.concatenate(parts, axis=1)          # [C, 64]
        pools.append(np.ascontiguousarray(full.T).astype(np.float32))
    return tuple(pools)



# revision 29
# speedup vs baseline: 1.0294x; 1.0294x over previous
"""Trainium2 Bass kernel for nn_CGT_21354577396059 (GPS-style GNN, 3 streams x 3 layers).

Strategy (8 NeuronCores, SPMD):
- Node-shard: core c owns nodes [2048c, 2048c+2048) = 8 graphs of 256 nodes.
- Activations feature-major in SBUF: hT [C=128 partitions, 2048 nodes] fp32,
  bf16 copies as matmul inputs (fp32 matmul is 4x slower on the PE).
- GIN segment_sum: edges dst-sorted per core, padded per 128-node window to a
  uniform chunk capacity; src rows gathered from a bf16 node-major DRAM table
  (gpsimd dma_gather, 256B rows); scatter via one-hot matmuls
  aggT += gathered_chunk.T @ onehot_chunk (PE, fp32 PSUM accumulate).
- The bf16 node table is rebuilt each layer via PE transpose + 8-core AllGather.
- Attention is graph-local: scoresT = kT.T @ qT per (graph, head, key-chunk)
  with row-packed tile_position; exp on ACT (no max subtraction; |scores|<~50);
  softmax sums via col-tiled ones-matmuls; o via col-tiled matmuls contracted
  over keys; normalization with a DVE reciprocal + elementwise mul.

kernel(**inputs) takes the FULL unsharded inputs and returns
(pool(h0), pool(ha), pool(hb)) — tuple of [64, 128] float32 — like the reference.
"""
import sys
import numpy as np
import ml_dtypes

if "/opt/trn_rl_repo" not in sys.path:
    sys.path.insert(0, "/opt/trn_rl_repo")

import os
import concourse.bass as bass  # noqa: F401
import concourse.tile as tile
from concourse import bacc, mybir, library_config
from concourse.bass_utils import run_bass_kernel_spmd

BF = ml_dtypes.bfloat16

# Problem constants (self-contained; no reads of /root/problem/*)
N_NODES = 16384
N_GRAPHS = 64
NPG = 256
FEA_DIM = 32
PE_DIM = 20
C = 128
HEADS = 4
HD = C // HEADS
L = 3
BN_EPS = 1e-5
S_BN = float(1.0 / np.sqrt(1.0 + BN_EPS))

N_CORES = 8
NPC = N_NODES // N_CORES   # 2048
NG_C = NPC // NPG          # 8 graphs per core
WIN = 128
NWIN = NPC // WIN          # 16
NCHUNK = NPC // 128        # 16
WGRP = 2                   # windows per dma_gather call

fdt = mybir.dt.float32
bdt = mybir.dt.bfloat16
i16 = mybir.dt.int16
AF = mybir.ActivationFunctionType
AX = mybir.AxisListType
ALU = mybir.AluOpType
AG_GROUPS = [list(range(N_CORES))]


# ---------------------------------------------------------------------------
# Host-side data prep
# ---------------------------------------------------------------------------

def _wrap_idxs(idx):
    """dma_gather idx layout [128, n/16] int16: idx i at (i%16, i//16),
    replicated across the 8 16-partition blocks."""
    n = len(idx)
    a = np.asarray(idx, np.int16).reshape(n // 16, 16).T
    return np.ascontiguousarray(np.tile(a, (8, 1)))


def _prep_edges_stream(edge_index):
    """Returns (cap_chunks, [(gidx_wrapped, dstv_bf16)] per core).

    Edges within each 128-dst window are sorted by src for HBM row-buffer
    locality during the gather. dstv[p, w*cap+t] is the window-local dst of
    edge slot t*128+p in window w (-1 for padding); the one-hot scatter
    matrices are generated on-chip from it."""
    src = np.asarray(edge_index[0]).astype(np.int64)
    dst = np.asarray(edge_index[1]).astype(np.int64)
    per_core_wins = []
    max_w = 0
    for c in range(N_CORES):
        m = (dst >= c * NPC) & (dst < (c + 1) * NPC)
        s, d = src[m], dst[m] - c * NPC
        order = np.argsort(d, kind="stable")
        s, d = s[order], d[order]
        wins = []
        for w in range(NWIN):
            mw = (d >= w * WIN) & (d < (w + 1) * WIN)
            sw, dw = s[mw], d[mw] - w * WIN
            so = np.argsort(sw, kind="stable")
            wins.append((sw[so], dw[so]))
            max_w = max(max_w, int(mw.sum()))
        per_core_wins.append(wins)
    cap_e = ((max_w + 127) // 128) * 128     # edges per window, padded
    cap = cap_e // 128
    out = []
    for c in range(N_CORES):
        srcs = np.zeros(NWIN * cap_e, np.int64)
        dstv = np.full((NWIN * cap_e,), -1.0, np.float32)
        for w in range(NWIN):
            s, dloc = per_core_wins[c][w]
            n = len(s)
            srcs[w * cap_e:w * cap_e + n] = s
            dstv[w * cap_e + np.arange(n)] = dloc
        dv = dstv.reshape(NWIN, cap, 128).transpose(2, 0, 1).reshape(128, NWIN * cap)
        out.append((_wrap_idxs(srcs), np.ascontiguousarray(dv.astype(BF))))
    return cap, out


def _pack_host(inputs):
    inp = {k: np.asarray(v) for k, v in inputs.items()}
    rt2 = 1.0 / np.sqrt(HD)

    blocks, offs = [], {}

    def add(name, arr):
        arr = np.asarray(arr, np.float32)
        k, m = arr.shape
        buf = np.zeros((128, m), BF)
        buf[:k] = arr.astype(BF)
        offs[name] = sum(b.shape[1] for b in blocks)
        blocks.append(buf)

    add("emb", inp["node_emb_w"])
    add("pe", inp["pe_lin_w"])
    add("I2", 2.0 * np.eye(C))       # h2 fold: ACT scale s gives 2s*h
    add("I1", np.eye(C))             # transpose identity + acc fold
    add("ones32", np.ones((C, HD)))
    add("iota", np.tile(np.arange(C, dtype=np.float32)[None, :], (C, 1)))
    for l in range(L):
        aw = inp["attn_in_w"][l]
        add(f"gw1_{l}", inp["gin_w1"][l])
        add(f"gw2_{l}", inp["gin_w2"][l])
        add(f"qT_{l}", (aw[0:C] * rt2).T)
        for h in range(HEADS):
            km = aw[C:2 * C].T.copy()
            mask = np.zeros(C); mask[32 * h:32 * h + 32] = 1.0
            add(f"kTm{h}_{l}", km * mask[None, :])
        add(f"vT_{l}", aw[2 * C:3 * C].T)
        add(f"ow_{l}", inp["attn_out_w"][l].T)
        add(f"m1_{l}", inp["mlp_w1"][l])
        add(f"m2a_{l}", inp["mlp_w2"][l][0:C])
        add(f"m2b_{l}", inp["mlp_w2"][l][C:2 * C])
    wts = np.ascontiguousarray(np.concatenate(blocks, axis=1))

    bvecs, boffs = [], {}

    def addb(name, vec):
        vec = np.asarray(vec, np.float32).reshape(-1)
        assert vec.shape == (C,)
        boffs[name] = len(bvecs)
        bvecs.append(vec)

    addb("eb", inp["node_emb_b"] + inp["pe_lin_b"])
    for l in range(L):
        ab = inp["attn_in_b"][l]
        addb(f"gb1_{l}", inp["gin_b1"][l])
        addb(f"sgb2_{l}", S_BN * inp["gin_b2"][l])
        addb(f"qb_{l}", ab[0:C] * rt2)
        for h in range(HEADS):
            mask = np.zeros(C); mask[32 * h:32 * h + 32] = 1.0
            addb(f"kbm{h}_{l}", ab[C:2 * C] * mask)
        addb(f"sob_{l}", S_BN * inp["attn_out_b"][l])
        addb(f"m1ba_{l}", inp["mlp_b1"][l][0:C])
        addb(f"m1bb_{l}", inp["mlp_b1"][l][C:2 * C])
        addb(f"smb2_{l}", S_BN * inp["mlp_b2"][l])
    biases = np.ascontiguousarray(np.stack(bvecs, axis=1).astype(np.float32))

    vbias = np.ascontiguousarray(np.stack(
        [np.tile(inp["attn_in_b"][l][2 * C:3 * C], (128, 1)) for l in range(L)]
    ).astype(np.float32))

    caps, edges = [], []
    for key in ("edge_index", "edge_index1", "edge_index2"):
        cap_chunks, per_core = _prep_edges_stream(inp[key])
        caps.append(cap_chunks)
        edges.append(per_core)

    xs = [inp["x"], inp["x1"], inp["x2"]]
    in_maps = []
    for c in range(N_CORES):
        m = {"wts": wts, "biases": biases, "vbias": vbias}
        sl = slice(c * NPC, (c + 1) * NPC)
        for s in range(3):
            m[f"xT{s}"] = np.ascontiguousarray(xs[s][sl].T.astype(BF))
            m[f"gidx{s}"] = edges[s][c][0]
            m[f"dstv{s}"] = edges[s][c][1]
        m["peT"] = np.ascontiguousarray(inp["pe"][sl].T.astype(BF))
        in_maps.append(m)

    return in_maps, caps, offs, boffs, wts.shape[1], biases.shape[1]


# ---------------------------------------------------------------------------
# Kernel builder
# ---------------------------------------------------------------------------


def _mm(nc, out, lhsT, rhs, start, stop, nmax=512):
    """matmul with moving free dim split to <=512 (ISA limit)."""
    n = rhs.shape[-1]
    assert out.shape[-1] == n
    for i in range(0, n, nmax):
        j = min(i + nmax, n)
        nc.tensor.matmul(out[:, i:j], lhsT, rhs[:, i:j], start=start, stop=stop)


def build_kernel(caps, offs, boffs, wcols, bcols, n_layers=L, n_streams=3,
                 dbg_stream=None):
    nc = bacc.Bacc("TRN2", target_bir_lowering=False, num_devices=N_CORES,
                   num_swdge_queues=4)

    wts_d = nc.dram_tensor("wts", [128, wcols], bdt, kind="ExternalInput")
    bias_d = nc.dram_tensor("biases", [128, bcols], fdt, kind="ExternalInput")
    vbias_d = nc.dram_tensor("vbias", [L, 128, 128], fdt, kind="ExternalInput")
    peT_d = nc.dram_tensor("peT", [PE_DIM, NPC], bdt, kind="ExternalInput")
    xT_d, gidx_d, dstv_d = [], [], []
    for s in range(3):
        cap = caps[s]
        xT_d.append(nc.dram_tensor(f"xT{s}", [FEA_DIM, NPC], bdt,
                                   kind="ExternalInput"))
        gidx_d.append(nc.dram_tensor(f"gidx{s}", [128, NWIN * cap * 8], i16,
                                     kind="ExternalInput"))
        dstv_d.append(nc.dram_tensor(f"dstv{s}", [128, NWIN * cap], bdt,
                                     kind="ExternalInput"))
    pool_out = nc.dram_tensor("pool_out", [3, C, NG_C], fdt,
                              kind="ExternalOutput")
    dbg_out = None
    if dbg_stream is not None:
        dbg_out = nc.dram_tensor("dbg_out", [C, NPC], fdt, kind="ExternalOutput")

    with tile.TileContext(nc) as tc:
        with (
            tc.tile_pool(name="const", bufs=1) as const_p,
            tc.tile_pool(name="hstate", bufs=1) as hstate_p,
            tc.tile_pool(name="big", bufs=1) as big_p,       # full-width tiles
            tc.tile_pool(name="chunk", bufs=2) as chunk_p,   # [C,1024]-ish tiles
            tc.tile_pool(name="gath", bufs=2) as gath_p,
            tc.tile_pool(name="ohp", bufs=2) as oh_p,
            tc.tile_pool(name="ps2", bufs=3, space="PSUM") as ps2,   # 2-bank
            tc.tile_pool(name="ps1", bufs=2, space="PSUM") as ps1,   # 1-bank
            tc.tile_pool(name="dram", bufs=4, space="DRAM") as dram_p,
        ):
            wts = const_p.tile([128, wcols], bdt)
            nc.sync.dma_start(wts[:], wts_d[:])
            bia = const_p.tile([128, bcols], fdt)
            nc.sync.dma_start(bia[:], bias_d[:])
            vbia = const_p.tile([128, L, 128], fdt)
            nc.sync.dma_start(vbia[:], vbias_d[:].rearrange("l p c -> p l c"))

            def W(name, width=128, rows=128):
                return wts[0:rows, offs[name]:offs[name] + width]

            def B(name):
                return bia[:, boffs[name]:boffs[name] + 1]

            hT = [hstate_p.tile([C, NPC], fdt, tag=f"hT{s}", name=f"hT{s}")
                  for s in range(3)]
            hB = [hstate_p.tile([C, NPC], bdt, tag=f"hB{s}", name=f"hB{s}")
                  for s in range(3)]
            gidx = []
            dstv = []
            for s in range(n_streams):
                t = hstate_p.tile([128, NWIN * caps[s] * 8], i16,
                                  tag=f"gidx{s}", name=f"gidx{s}")
                nc.sync.dma_start(t[:], gidx_d[s][:])
                gidx.append(t)
                dv = hstate_p.tile([128, NWIN * caps[s]], bdt,
                                   tag=f"dstv{s}", name=f"dstv{s}")
                nc.sync.dma_start(dv[:], dstv_d[s][:])
                dstv.append(dv)

            def emit_table(src_bf, s):
                """feature-major bf16 [C, NPC] -> node-major -> AllGather table."""
                nm = big_p.tile([128, NCHUNK, C], bdt, tag="nm")
                for k in range(NCHUNK):
                    pt = ps1.tile([128, 128], bdt, tag="ps1", name="pt")
                    nc.tensor.transpose(pt[:], src_bf[:, k * 128:(k + 1) * 128],
                                        W("I1"))
                    nc.vector.tensor_copy(nm[:, k, :], pt[:])
                agi = dram_p.tile([NPC, C], bdt, tag="agi")
                nc.sync.dma_start(agi[:].rearrange("(k p) c -> p k c", p=128),
                                  nm[:])
                tab = dram_p.tile([N_NODES, C], bdt, tag=f"tab{s}", name="tab",
                                  bufs=1, addr_space="Shared")
                nc.gpsimd.collective_compute(
                    "AllGather", ALU.bypass, replica_groups=AG_GROUPS,
                    ins=[agi.opt()], outs=[tab.opt()])
                return tab

            # ---------------- embedding ----------------
            tabs = [None] * 3
            for s in range(n_streams):
                xT = chunk_p.tile([FEA_DIM, NPC], bdt, tag="xT", bufs=1)
                nc.sync.dma_start(xT[:], xT_d[s][:])
                peT = chunk_p.tile([PE_DIM, NPC], bdt, tag="peT", bufs=1)
                nc.sync.dma_start(peT[:], peT_d[:])
                for ch in range(2):
                    sl = slice(ch * 1024, (ch + 1) * 1024)
                    ep = ps2.tile([C, 1024], fdt, tag="ps2")
                    _mm(nc, ep, W("emb", rows=FEA_DIM), xT[:, sl],
                        start=True, stop=False)
                    _mm(nc, ep, W("pe", rows=PE_DIM), peT[:, sl],
                        start=False, stop=True)
                    nc.scalar.activation(hT[s][:, sl], ep[:], AF.Identity,
                                         bias=B("eb"))
                    nc.vector.tensor_copy(hB[s][:, sl], hT[s][:, sl])
                tabs[s] = emit_table(hB[s], s)

            # ---------------- layers ----------------
            for l in range(n_layers):
                for s in range(n_streams):
                    _gps_layer(nc, l, caps[s], hT[s], hB[s], tabs, s,
                               gidx[s], dstv[s], W, B, vbia,
                               big_p, chunk_p, gath_p, oh_p, ps2, ps1,
                               emit_table, last=(l == n_layers - 1))

            # ---------------- pooling ----------------
            for s in range(n_streams):
                po = chunk_p.tile([C, NG_C], fdt, tag="pool")
                nc.vector.reduce_sum(
                    out=po[:],
                    in_=hT[s][:].rearrange("c (g n) -> c g n", g=NG_C),
                    axis=AX.X)
                nc.sync.dma_start(pool_out[s], po[:])

            if dbg_stream is not None:
                nc.sync.dma_start(dbg_out[:], hT[dbg_stream][:])

    nc.compile()
    return nc


def _gps_layer(nc, l, cap, hT, hB, tabs, s, gidx, dstv, W, B, vbia,
               big_p, chunk_p, gath_p, oh_p, ps2, ps1, emit_table, last):
    AFI = AF.Identity
    K_ATTN = True
    K_GIN = True
    K_GATHER = True
    K_SCORES = True
    K_SUMS = True
    K_OV = True

    # ---------------- attention (graph-local) ----------------
    if not K_ATTN:
        o_bf = big_p.tile([C, NPC], bdt, tag="o_bf")
        nc.vector.memset(o_bf[:], 0.0)
    qT = big_p.tile([C, NPC], bdt, tag="qT", name="qT") if K_ATTN else None
    for ch in range(2 if K_ATTN else 0):
        sl = slice(ch * 1024, (ch + 1) * 1024)
        qp = ps2.tile([C, 1024], fdt, tag="ps2")
        _mm(nc, qp, W(f"qT_{l}"), hB[:, sl], start=True, stop=True)
        nc.scalar.activation(qT[:, sl], qp[:], AFI, bias=B(f"qb_{l}"))
    v_sb = big_p.tile([128, NCHUNK, C], bdt, tag="v_sb", name="v_sb") if K_ATTN else None
    for k in range(NCHUNK if K_ATTN else 0):
        vp = ps1.tile([128, C], fdt, tag="ps1")
        nc.tensor.matmul(vp[:], hB[:, k * 128:(k + 1) * 128], W(f"vT_{l}"),
                         start=True, stop=True)
        nc.vector.tensor_add(v_sb[:, k, :], vp[:], vbia[:, l, :])

    expT = big_p.tile([128, 2, HEADS, NG_C, NPG], bdt, tag="expT", name="expT") if K_ATTN else None
    if K_ATTN and not K_SCORES:
        nc.vector.memset(expT[:], 1.0)
    for h in range(HEADS if (K_ATTN and K_SCORES) else 0):
        kTm = big_p.tile([C, NPC], bdt, tag="kTm", name="kTm")
        for ch in range(2):
            sl = slice(ch * 1024, (ch + 1) * 1024)
            kp = ps2.tile([C, 1024], fdt, tag="ps2")
            _mm(nc, kp, W(f"kTm{h}_{l}"), hB[:, sl], start=True, stop=True)
            nc.scalar.activation(kTm[:, sl], kp[:], AFI, bias=B(f"kbm{h}_{l}"))
        for kc in range(2):
            for gh in range(2):
                sp = ps2.tile([128, 4, NPG], fdt, tag="ps2")
                for gi in range(4):
                    g = gh * 4 + gi
                    ksl = kTm[:, g * NPG + kc * 128:g * NPG + kc * 128 + 128]
                    qsl = qT[:, g * NPG:(g + 1) * NPG]
                    nc.tensor.matmul(sp[:, gi, :], ksl, qsl,
                                     start=True, stop=True)
                nc.scalar.activation(expT[:, kc, h, gh * 4:gh * 4 + 4, :],
                                     sp[:], AF.Exp)

    recip = big_p.tile([128, NPC], bdt, tag="recip", name="recip") if K_ATTN else None
    if K_ATTN and not K_SUMS:
        nc.vector.memset(recip[:], 1.0 / 256.0)
    for half in range(2 if (K_ATTN and K_SUMS) else 0):
        sm = ps2.tile([128, 1024], fdt, tag="ps2")
        for h in range(HEADS):
            for qc in range(2):
                osl = sm[32 * h:32 * h + 32, qc * 512:(qc + 1) * 512]
                for kc in range(2):
                    rhs = expT[:, kc, h, :, :].rearrange("p g q -> p (g q)")
                    nc.tensor.matmul(
                        osl, W("ones32", width=HD),
                        rhs[:, half * 1024 + qc * 512:half * 1024 + (qc + 1) * 512],
                        start=(kc == 0), stop=(kc == 1),
                        tile_position=(0, 32 * h))
        with nc.allow_low_precision(reason="softmax recip bf16 ok at 2e-2 gate"):
            nc.vector.reciprocal(recip[:, half * 1024:(half + 1) * 1024], sm[:])

    if K_ATTN:
        o_bf = big_p.tile([C, NPC], bdt, tag="o_bf", name="o_bf")
        if not K_OV:
            nc.vector.memset(o_bf[:], 0.0)
    for g in range(NG_C if (K_ATTN and K_OV) else 0):
        op = ps1.tile([128, NPG], fdt, tag="ps1")
        for h in range(HEADS):
            for kc in range(2):
                lhs = v_sb[:, g * 2 + kc, 32 * h:32 * h + 32]
                rhs = expT[:, kc, h, g, :]
                nc.tensor.matmul(op[32 * h:32 * h + 32, :], lhs, rhs,
                                 start=(kc == 0), stop=(kc == 1),
                                 tile_position=(0, 32 * h))
        nc.vector.tensor_mul(o_bf[:, g * NPG:(g + 1) * NPG], op[:],
                             recip[:, g * NPG:(g + 1) * NPG])

    # ---------------- GIN ----------------
    g_bf = big_p.tile([C, NPC], bdt, tag="g_bf")
    if not K_GIN:
        nc.vector.tensor_copy(g_bf[:], hT[:])
    tab = tabs[s]
    cap_e = cap * 128                       # edges per window
    GMAX = 512    # half the 1024-desc SWDGE ring per call so desc-gen
                  # pipelines with ring drain instead of stalling
    qn = (l * 3 + s) % 4
    for wg in range(NWIN // WGRP if K_GIN else 0):
        nidx = WGRP * cap_e
        gath = gath_p.tile([128, WGRP * cap, C], bdt, tag="gath")
        base = wg * nidx
        if K_GATHER:
            off = 0
            while off < nidx:
                n = min(GMAX, nidx - off)
                isl = gidx[:, (base + off) // 16:(base + off + n) // 16]
                nc.gpsimd.dma_gather(
                    gath[:, off // 128:(off + n) // 128, :], tab[:], isl,
                    n, n, C, queue_num=qn)
                qn = (qn + 1) % 4
                off += n
        else:
            nc.vector.memset(gath[:], 0.0)
        for wi in range(WGRP):
            w = wg * WGRP + wi
            ohs = oh_p.tile([128, cap, WIN], bdt, tag="ohs")
            io = W("iota")
            b0 = bass.AP(io.tensor, io.offset,
                         [list(io.ap[0]), [0, cap], list(io.ap[1])])
            dv = dstv[:, w * cap:(w + 1) * cap]
            b1 = bass.AP(dv.tensor, dv.offset,
                         [list(dv.ap[0]), list(dv.ap[1]), [0, WIN]])
            nc.vector.tensor_tensor(ohs[:], b0, b1, ALU.is_equal)
            ap = ps1.tile([C, WIN], fdt, tag="ps1")
            for t in range(cap):
                nc.tensor.matmul(ap[:], gath[:, wi * cap + t, :], ohs[:, t, :],
                                 start=(t == 0), stop=(t == cap - 1))
            nc.vector.tensor_add(g_bf[:, w * WIN:(w + 1) * WIN], ap[:],
                                 hT[:, w * WIN:(w + 1) * WIN])

    # GIN MLP + combine with attention: acc = h1 + h2
    acc_bf = big_p.tile([C, NPC], bdt, tag="acc_bf")
    r_bf = big_p.tile([C, NPC], bdt, tag="r_bf")
    for ch in range(2):
        sl = slice(ch * 1024, (ch + 1) * 1024)
        tp = ps2.tile([C, 1024], fdt, tag="ps2")
        _mm(nc, tp, W(f"gw1_{l}"), g_bf[:, sl], start=True, stop=True)
        nc.scalar.activation(r_bf[:, sl], tp[:], AF.Relu, bias=B(f"gb1_{l}"))
        up = ps2.tile([C, 1024], fdt, tag="ps2")
        _mm(nc, up, W(f"gw2_{l}"), r_bf[:, sl], start=True, stop=True)
        h1 = chunk_p.tile([C, 1024], bdt, tag="h1")
        nc.scalar.activation(h1[:], up[:], AFI, bias=B(f"sgb2_{l}"), scale=S_BN)
        ap2 = ps2.tile([C, 1024], fdt, tag="ps2")
        _mm(nc, ap2, W(f"ow_{l}"), o_bf[:, sl], start=True, stop=False)
        _mm(nc, ap2, W("I2"), hB[:, sl], start=False, stop=True)
        h2 = chunk_p.tile([C, 1024], bdt, tag="h2")
        nc.scalar.activation(h2[:], ap2[:], AFI, bias=B(f"sob_{l}"), scale=S_BN)
        nc.vector.tensor_add(acc_bf[:, sl], h1[:], h2[:])

    # MLP (chunked: r2 hidden kept per-1024-chunk to save SBUF)
    for ch in range(2):
        sl = slice(ch * 1024, (ch + 1) * 1024)
        r2c = chunk_p.tile([C, 2, 1024], bdt, tag="r2c", bufs=1)
        for mh in range(2):
            mp = ps2.tile([C, 1024], fdt, tag="ps2")
            _mm(nc, mp, W(f"m1_{l}", width=256)[:, mh * 128:(mh + 1) * 128],
                acc_bf[:, sl], start=True, stop=True)
            bname = f"m1ba_{l}" if mh == 0 else f"m1bb_{l}"
            nc.scalar.activation(r2c[:, mh, :], mp[:], AF.Relu, bias=B(bname))
        m2p = ps2.tile([C, 1024], fdt, tag="ps2")
        _mm(nc, m2p, W(f"m2a_{l}"), r2c[:, 0, :], start=True, stop=False)
        _mm(nc, m2p, W(f"m2b_{l}"), r2c[:, 1, :], start=False, stop=False)
        _mm(nc, m2p, W("I1"), acc_bf[:, sl], start=False, stop=True)
        dh = chunk_p.tile([C, 1024], bdt, tag="dh")
        nc.scalar.activation(dh[:], m2p[:], AFI, bias=B(f"smb2_{l}"), scale=S_BN)
        nc.vector.tensor_add(hT[:, sl], hT[:, sl], dh[:])
        nc.vector.tensor_copy(hB[:, sl], hT[:, sl])

    if not last:
        tabs[s] = emit_table(hB, s)


# ---------------------------------------------------------------------------
# Entry point
# ---------------------------------------------------------------------------

_CACHE = {}


def _get_kernel(caps, offs, boffs, wcols, bcols, **kw):
    key = (tuple(caps), wcols, bcols, tuple(sorted(kw.items())))
    if key not in _CACHE:
        _CACHE[key] = build_kernel(caps, offs, boffs, wcols, bcols, **kw)
    return _CACHE[key]


def kernel(**inputs):
    in_maps, caps, offs, boffs, wcols, bcols = _pack_host(inputs)
    nc = _get_kernel(caps, offs, boffs, wcols, bcols)
    res = run_bass_kernel_spmd(nc, in_maps, core_ids=list(range(N_CORES)))
    pools = []
    for si in range(3):
        parts = [np.asarray(res.results[c]["pool_out"][si])
                 for c in range(N_CORES)]
        full = np.concatenate(parts, axis=1)          # [C, 64]
        pools.append(np.ascontiguousarray(full.T).astype(np.float32))
    return tuple(pools)

